# revision 1
# baseline (speedup 1.0000x reference)
"""Trainium2 Bass kernel for nn_GRU_77025943486969.

Reference computation (see problem spec):
  2-layer GRU (B=256, T=128, N=H=512)  ->  fc  ->  silu  ->  softmax
  ->  per-sample iterative water-filling (clip to [0, 0.1], redistribute).

Strategy: data-parallel over batch across 8 NeuronCores (32 samples/core).
Per core everything runs in a transposed layout: hidden dim on the 128
partitions (4 chunks of 128), batch in the free dimension (32), so that the
gate elementwise math uses all 128 DVE/ACT lanes.  Matmuls run in bf16 with
fp32 PSUM accumulation; gate math and recurrent state stay fp32.

The input projections xi = x @ W_ih.T + b for both layers are computed as
wide (512-column) matmuls in 16-timestep blocks and software-pipelined
against the sequential recurrences so the PE always has independent work.
"""

import os
import numpy as np
import ml_dtypes

import concourse.bass as bass
import concourse.mybir as mybir
import concourse.tile as tile
from concourse.bass_utils import run_bass_kernel_spmd

BF = ml_dtypes.bfloat16
F32 = mybir.dt.float32
BF16 = mybir.dt.bfloat16
OP = mybir.AluOpType
AF = mybir.ActivationFunctionType

B, T, N, H = 256, 128, 512, 512
NCORES = 8
BS = B // NCORES            # 32 samples per core
KC = H // 128               # 4 contraction chunks
MC = 3 * H // 128           # 12 gate-row chunks (r: 0-3, z: 4-7, n: 8-11)
TB = 16                     # timesteps per pipeline block
NBLK = int(os.environ.get("GRU_NBLK", T // TB))  # dev knob; 8 = full
UB = 0.1
NITER_WF = 16               # water-filling rounds (>= 11 provably converged)
REPS = int(os.environ.get("GRU_REPS", "1"))  # repeat whole kernel (timing only)


def _split_sync(nc, max_waits=1, max_updates=1):
    """This container's walrus accepts only one sync wait per instruction.
    Move extra waits onto same-engine NoOps placed just before; move extra
    updates of compute instructions onto NoOps just after (engines complete
    in order).  DMA instructions must keep a single update (async completion)
    so assert they do."""
    for f in nc.m.functions:
        for bb in f.blocks:
            out = []
            changed = False
            for inst in bb.instructions:
                si = getattr(inst, "sync_info", None)
                pre, post = [], []
                if si is not None and si.on_wait and len(si.on_wait) > max_waits:
                    waits = list(si.on_wait)
                    extra, keep = waits[:-max_waits], waits[-max_waits:]
                    i = 0
                    while extra:
                        chunk, extra = extra[:max_waits], extra[max_waits:]
                        nop = mybir.InstNoOp(name=f"{inst.name}-ws{i}", ins=[], outs=[])
                        nop.engine = inst.engine
                        nop.sync_info = mybir.SyncInfo(on_wait=chunk, on_update=[])
                        pre.append(nop)
                        i += 1
                    inst.sync_info = mybir.SyncInfo(
                        on_wait=keep, on_update=list(si.on_update)
                    )
                    si = inst.sync_info
                if si is not None and si.on_update and len(si.on_update) > max_updates:
                    assert not isinstance(inst, mybir.InstTensorCopy) and (
                        "DMA" not in type(inst).__name__
                    ), f"multi-update DMA {inst.name} cannot be split"
                    ups = list(si.on_update)
                    keep_u, extra_u = ups[:max_updates], ups[max_updates:]
                    i = 0
                    while extra_u:
                        chunk, extra_u = extra_u[:max_updates], extra_u[max_updates:]
                        nop = mybir.InstNoOp(name=f"{inst.name}-us{i}", ins=[], outs=[])
                        nop.engine = inst.engine
                        nop.sync_info = mybir.SyncInfo(on_wait=[], on_update=chunk)
                        post.append(nop)
                        i += 1
                    inst.sync_info = mybir.SyncInfo(
                        on_wait=list(si.on_wait), on_update=keep_u
                    )
                if pre or post:
                    changed = True
                out.extend(pre)
                out.append(inst)
                out.extend(post)
            if changed:
                bb.instructions = out
    return nc


def _build():
    nc = bass.Bass()
    dp = nc.declare_dram_parameter
    xts_e = dp("xts", [NBLK, 128, KC, TB, BS], BF16, isOutput=False)
    wih0_e = dp("wih0", [128, KC, 3 * H], BF16, isOutput=False)
    whh0_e = dp("whh0", [128, KC, 3 * H], BF16, isOutput=False)
    wih1_e = dp("wih1", [128, KC, 3 * H], BF16, isOutput=False)
    whh1_e = dp("whh1", [128, KC, 3 * H], BF16, isOutput=False)
    fcw_e = dp("fcw", [128, KC, N], F32, isOutput=False)
    bias0_e = dp("bias0", [128, MC], F32, isOutput=False)  # ACT drain bias, layer 0
    bias1_e = dp("bias1", [128, MC], F32, isOutput=False)
    bhn0_e = dp("bhn0", [128, KC, BS], F32, isOutput=False)  # b_hh0 n-part bcast
    bhn1_e = dp("bhn1", [128, KC, BS], F32, isOutput=False)
    fcb_e = dp("fcb", [BS, N], F32, isOutput=False)  # fc bias replicated per row
    y_e = dp("y", [BS, N], F32, isOutput=True)

    with tile.TileContext(nc) as tc:
        with (
            tc.tile_pool(name="wpool", bufs=1) as wp,
            tc.tile_pool(name="xpool", bufs=3) as xp,
            tc.tile_pool(name="hseq", bufs=3) as hp,
            tc.tile_pool(name="xipool", bufs=4) as xip,
            tc.tile_pool(name="state", bufs=2) as sp,
            tc.tile_pool(name="gates", bufs=2) as gp,
            tc.tile_pool(name="head", bufs=2) as hd,
            tc.tile_pool(name="ps_xi", bufs=2, space="PSUM") as ps_xi,
            tc.tile_pool(name="ps_r0", bufs=2, space="PSUM") as ps_r0,
            tc.tile_pool(name="ps_r1", bufs=2, space="PSUM") as ps_r1,
            tc.tile_pool(name="ps_fc", bufs=1, space="PSUM") as ps_fc,
        ):
            # ---- resident weights/constants -------------------------------
            wih0 = wp.tile([128, KC, 3 * H], BF16)
            nc.sync.dma_start(wih0[:], wih0_e[:])
            whh0 = wp.tile([128, KC, 3 * H], BF16)
            nc.sync.dma_start(whh0[:], whh0_e[:])
            wih1 = wp.tile([128, KC, 3 * H], BF16)
            nc.sync.dma_start(wih1[:], wih1_e[:])
            whh1 = wp.tile([128, KC, 3 * H], BF16)
            nc.sync.dma_start(whh1[:], whh1_e[:])
            fcw = wp.tile([128, KC, N], F32)
            nc.sync.dma_start(fcw[:], fcw_e[:])
            bias0 = wp.tile([128, MC], F32)
            nc.sync.dma_start(bias0[:], bias0_e[:])
            bias1 = wp.tile([128, MC], F32)
            nc.sync.dma_start(bias1[:], bias1_e[:])
            bhn0 = wp.tile([128, KC, BS], F32)
            nc.sync.dma_start(bhn0[:], bhn0_e[:])
            bhn1 = wp.tile([128, KC, BS], F32)
            nc.sync.dma_start(bhn1[:], bhn1_e[:])
            fcb = wp.tile([BS, N], F32)
            nc.sync.dma_start(fcb[:], fcb_e[:])
            zrhs = wp.tile([128, KC, BS], BF16)
            nc.vector.memset(zrhs[:], 0.0)

            for rep in range(REPS):
                h0f = sp.tile([128, KC, BS], F32, tag="h0f", name=f"h0f_{rep}")
                nc.vector.memset(h0f[:], 0.0)
                h1f = sp.tile([128, KC, BS], F32, tag="h1f", name=f"h1f_{rep}")
                nc.vector.memset(h1f[:], 0.0)

                xt_tiles = [None] * NBLK
                hs_tiles = [None] * NBLK
                xi_tiles = [[None] * NBLK, [None] * NBLK]
                hstate = [h0f, h1f]
                h1b_box = [None]

                def load_x_block(c):
                    xt = xp.tile([128, KC, TB, BS], BF16, tag="xt")
                    nc.sync.dma_start(xt[:], xts_e[c])
                    xt_tiles[c] = xt

                def queue_xi_block(layer, c, tasks):
                    """Queue the 12 m-chunk matmuls of xi[layer][c]."""
                    xi = xip.tile(
                        [128, TB, MC, BS], BF16, tag="xi", name=f"xi{layer}_{c}_{rep}"
                    )
                    xi_tiles[layer][c] = xi
                    w = wih0 if layer == 0 else wih1
                    bias = bias0 if layer == 0 else bias1
                    rhs = xt_tiles[c] if layer == 0 else hs_tiles[c]

                    def chunk(m):
                        acc = ps_xi.tile([128, TB * BS], F32, tag="psxi")
                        for k in range(KC):
                            nc.tensor.matmul(
                                acc[:],
                                w[:, k, 128 * m : 128 * (m + 1)],
                                rhs[:, k, :, :],
                                start=(k == 0),
                                stop=(k == KC - 1),
                            )
                        nc.scalar.activation(
                            xi[:, :, m, :],
                            acc.rearrange("p (t b) -> p t b", b=BS),
                            AF.Identity,
                            bias=bias[:, m : m + 1],
                        )

                    for m in range(MC):
                        tasks.append(((layer, c), lambda m=m: chunk(m)))

                def rec_step(layer, c, ti, t):
                    whh = whh0 if layer == 0 else whh1
                    bhn = bhn0 if layer == 0 else bhn1
                    xi = xi_tiles[layer][c]
                    psp = ps_r0 if layer == 0 else ps_r1
                    hf = hstate[layer]
                    if layer == 0:
                        if t == 0:
                            rsl = lambda k: zrhs[:, k, :]
                        elif ti == 0:
                            prev = hs_tiles[c - 1]
                            rsl = lambda k: prev[:, k, TB - 1, :]
                        else:
                            cur = hs_tiles[c]
                            rsl = lambda k: cur[:, k, ti - 1, :]
                    else:
                        if t == 0:
                            rsl = lambda k: zrhs[:, k, :]
                        else:
                            hb = h1b_box[0]
                            rsl = lambda k: hb[:, k, :]

                    g = psp.tile([128, MC, BS], F32, tag=f"g{layer}")
                    for m in range(MC):
                        for k in range(KC):
                            nc.tensor.matmul(
                                g[:, m, :],
                                whh[:, k, 128 * m : 128 * (m + 1)],
                                rsl(k),
                                start=(k == 0),
                                stop=(k == KC - 1),
                            )
                    # gates (transposed layout, [128, *, BS])
                    prz = gp.tile([128, 8, BS], F32, tag="prz")
                    nc.vector.tensor_add(prz[:], g[:, 0:8, :], xi[:, ti, 0:8, :])
                    rz = gp.tile([128, 8, BS], F32, tag="rz")
                    nc.scalar.activation(rz[:], prz[:], AF.Sigmoid)
                    # zbar = 1 - z on ACT (off the DVE chain)
                    zb = gp.tile([128, KC, BS], F32, tag="zb")
                    nc.scalar.activation(zb[:], rz[:, 4:8, :], AF.Copy, scale=-1.0,
                                         bias=1.0)
                    zh = gp.tile([128, KC, BS], F32, tag="zh")
                    nc.vector.tensor_mul(zh[:], rz[:, 4:8, :], hf[:])
                    hn = gp.tile([128, KC, BS], F32, tag="hn")
                    nc.vector.tensor_add(hn[:], g[:, 8:12, :], bhn[:])
                    t1 = gp.tile([128, KC, BS], F32, tag="t1")
                    nc.vector.tensor_mul(t1[:], hn[:], rz[:, 0:4, :])
                    pn = gp.tile([128, KC, BS], F32, tag="pn")
                    nc.vector.tensor_add(pn[:], t1[:], xi[:, ti, 8:12, :])
                    n_t = gp.tile([128, KC, BS], F32, tag="n_t")
                    nc.scalar.activation(n_t[:], pn[:], AF.Tanh)
                    m1 = gp.tile([128, KC, BS], F32, tag="m1")
                    nc.vector.tensor_mul(m1[:], n_t[:], zb[:])
                    hf2 = sp.tile([128, KC, BS], F32, tag=f"h{layer}f")
                    nc.vector.tensor_add(hf2[:], m1[:], zh[:])
                    if layer == 0:
                        nc.vector.tensor_copy(hs_tiles[c][:, :, ti, :], hf2[:])
                    else:
                        h1b = sp.tile([128, KC, BS], BF16, tag="h1b")
                        nc.vector.tensor_copy(h1b[:], hf2[:])
                        h1b_box[0] = h1b
                    hstate[layer] = hf2

                # ---- step-interleaved pipelined schedule ------------------
                LAG = TB + 4
                tasks = []
                load_x_block(0)
                queue_xi_block(0, 0, tasks)
                while tasks:  # xi0 block 0 fully before the loop
                    tasks.pop(0)[1]()
                for i in range(T + LAG):
                    if i < T:
                        c, ti = divmod(i, TB)
                        if ti == 0:
                            if c + 1 < NBLK:
                                load_x_block(c + 1)
                                queue_xi_block(0, c + 1, tasks)
                            hs_tiles[c] = hp.tile(
                                [128, KC, TB, BS], BF16, tag="hs",
                                name=f"hs{c}_{rep}",
                            )
                        rec_step(0, c, ti, i)
                        if ti == TB - 1:
                            queue_xi_block(1, c, tasks)
                    j = i - LAG
                    if 0 <= j < T:
                        jc, jti = divmod(j, TB)
                        if jti == 0:
                            # everything this layer-1 block needs must be done
                            rest = [t for t in tasks if t[0] == (1, jc)]
                            tasks[:] = [t for t in tasks if t[0] != (1, jc)]
                            for _, fn in rest:
                                fn()
                        rec_step(1, jc, jti, j)
                    for _ in range(2):
                        if tasks:
                            tasks.pop(0)[1]()

                # ---- head: fc + silu + softmax + water-filling -----------
                h1f = hstate[1]
                lp = ps_fc.tile([BS, N], F32)
                for k in range(KC):
                    nc.tensor.matmul(
                        lp[:], h1f[:, k, :], fcw[:, k, :],
                        start=(k == 0), stop=(k == KC - 1),
                    )
                lg = hd.tile([BS, N], F32, tag="lg")
                nc.vector.tensor_add(lg[:], lp[:], fcb[:])
                sl = hd.tile([BS, N], F32, tag="sl")
                nc.scalar.activation(sl[:], lg[:], AF.Silu)
                mx = hd.tile([BS, 1], F32, tag="mx")
                nc.vector.reduce_max(mx[:], sl[:], axis=mybir.AxisListType.X)
                nmx = hd.tile([BS, 1], F32, tag="nmx")
                nc.vector.tensor_scalar_mul(nmx[:], mx[:], -1.0)
                ex = hd.tile([BS, N], F32, tag="ex")
                nc.scalar.activation(ex[:], sl[:], AF.Exp, bias=nmx[:])
                se = hd.tile([BS, 1], F32, tag="se")
                nc.vector.reduce_sum(se[:], ex[:], axis=mybir.AxisListType.X)
                rs = hd.tile([BS, 1], F32, tag="rs")
                nc.vector.reciprocal(rs[:], se[:])
                w = hd.tile([BS, N], F32, tag="w")
                nc.vector.tensor_scalar_mul(w[:], ex[:], rs[:])
                t0 = hd.tile([BS, 1], F32, tag="t0")
                nc.vector.reduce_sum(t0[:], w[:], axis=mybir.AxisListType.X)
                wc = hd.tile([BS, N], F32, tag="w")
                nc.vector.tensor_scalar_min(wc[:], w[:], UB)
                for _ in range(NITER_WF):
                    noms = hd.tile([BS, N], F32, tag="noms")
                    s_n = hd.tile([BS, 1], F32, tag="s_n")
                    nc.vector.scalar_tensor_tensor(
                        noms[:], wc[:], UB, wc[:], OP.is_lt, OP.mult,
                        accum_out=s_n[:],
                    )
                    swc = hd.tile([BS, 1], F32, tag="swc")
                    nc.vector.reduce_sum(swc[:], wc[:], axis=mybir.AxisListType.X)
                    lft = hd.tile([BS, 1], F32, tag="lft")
                    nc.vector.tensor_scalar(
                        lft[:], swc[:], -1.0, t0[:], OP.mult, OP.add
                    )
                    rsn = hd.tile([BS, 1], F32, tag="rsn")
                    nc.vector.reciprocal(rsn[:], s_n[:])
                    gg = hd.tile([BS, 1], F32, tag="gg")
                    nc.vector.tensor_mul(gg[:], lft[:], rsn[:])
                    w2 = hd.tile([BS, N], F32, tag="noms2")
                    nc.vector.scalar_tensor_tensor(
                        w2[:], noms[:], gg[:], wc[:], OP.mult, OP.add
                    )
                    wc = hd.tile([BS, N], F32, tag="w")
                    nc.vector.tensor_scalar_min(wc[:], w2[:], UB)
                nc.sync.dma_start(y_e[:], wc[:])

    _split_sync(nc)
    return nc


def _prep_inputs(x, W_ih0, W_hh0, b_ih0, b_hh0, W_ih1, W_hh1, b_ih1, b_hh1,
                 fc_w, fc_b):
    """Host-side layout prep: transpose/shard/cast; returns per-core in_maps."""
    def wT(w):  # [3H, in] -> [128, KC, 3H] bf16 (lhsT chunks)
        wt = np.ascontiguousarray(w.T.reshape(KC, 128, 3 * H).transpose(1, 0, 2))
        return wt.astype(BF)

    def bias_comb(b_ih, b_hh):  # rz rows: both; n rows: b_ih only
        b = b_ih.astype(np.float64) + np.concatenate(
            [b_hh[: 2 * H], np.zeros(H)]
        )
        return np.ascontiguousarray(
            b.astype(np.float32).reshape(MC, 128).T
        )  # [128, MC]

    def bhn(b_hh):  # n-part [H] -> [128, KC, BS] broadcast over batch
        v = b_hh[2 * H :].astype(np.float32).reshape(KC, 128).T  # [128, KC]
        return np.ascontiguousarray(
            np.broadcast_to(v[:, :, None], (128, KC, BS))
        )

    fcw = np.ascontiguousarray(
        fc_w.T.reshape(KC, 128, N).transpose(1, 0, 2).astype(np.float32)
    )
    fcb = np.ascontiguousarray(np.broadcast_to(fc_b[None, :], (BS, N)).astype(np.float32))

    shared = {
        "wih0": wT(W_ih0), "whh0": wT(W_hh0),
        "wih1": wT(W_ih1), "whh1": wT(W_hh1),
        "fcw": fcw,
        "bias0": bias_comb(b_ih0, b_hh0), "bias1": bias_comb(b_ih1, b_hh1),
        "bhn0": bhn(b_hh0), "bhn1": bhn(b_hh1),
        "fcb": fcb,
    }
    in_maps = []
    for core in range(NCORES):
        xb = x[core * BS : (core + 1) * BS].astype(BF)  # [BS, T, N]
        # [N, T, BS] -> [KC, 128, NBLK, TB, BS] -> [NBLK, 128, KC, TB, BS]
        xt = xb.transpose(2, 1, 0).reshape(KC, 128, T // TB, TB, BS)
        xt = np.ascontiguousarray(xt.transpose(2, 1, 0, 3, 4))[:NBLK]
        m = dict(shared)
        m["xts"] = np.ascontiguousarray(xt)
        in_maps.append(m)
    return in_maps


_NC_CACHE = {}


def _get_nc():
    if "nc" not in _NC_CACHE:
        _NC_CACHE["nc"] = _build()
    return _NC_CACHE["nc"]


def kernel(**inputs):
    nc = _get_nc()
    in_maps = _prep_inputs(**{k: np.asarray(v) for k, v in inputs.items()})
    res = run_bass_kernel_spmd(nc, in_maps, list(range(NCORES)))
    return np.concatenate([res.results[i]["y"] for i in range(NCORES)], axis=0)


if __name__ == "__main__":
    rng = np.random.default_rng(0)
    ins = {
        "x": rng.standard_normal((B, T, N), dtype=np.float32),
        "W_ih0": rng.standard_normal((3 * H, N), dtype=np.float32) * 0.04,
        "W_hh0": rng.standard_normal((3 * H, H), dtype=np.float32) * 0.04,
        "b_ih0": rng.standard_normal(3 * H).astype(np.float32) * 0.04,
        "b_hh0": rng.standard_normal(3 * H).astype(np.float32) * 0.04,
        "W_ih1": rng.standard_normal((3 * H, H), dtype=np.float32) * 0.04,
        "W_hh1": rng.standard_normal((3 * H, H), dtype=np.float32) * 0.04,
        "b_ih1": rng.standard_normal(3 * H).astype(np.float32) * 0.04,
        "b_hh1": rng.standard_normal(3 * H).astype(np.float32) * 0.04,
        "fc_w": rng.standard_normal((N, H), dtype=np.float32) * 0.04,
        "fc_b": rng.standard_normal(N).astype(np.float32) * 0.04,
    }
    out = kernel(**ins)
    print("out", out.shape, out.dtype, out.sum())



# revision 22
# speedup vs baseline: 98.9568x; 98.9568x over previous
"""Trainium2 Bass kernel for nn_GRU_77025943486969.

Reference computation (see problem spec):
  2-layer GRU (B=256, T=128, N=H=512)  ->  fc  ->  silu  ->  softmax
  ->  per-sample iterative water-filling (clip to [0, 0.1], redistribute).

Strategy: data-parallel over batch across 8 NeuronCores (32 samples/core).
Per core everything runs in a transposed layout: hidden dim on the 128
partitions (4 chunks of 128), batch in the free dimension (32), so that the
gate elementwise math uses all 128 DVE/ACT lanes.  Matmuls run in bf16 with
fp32 PSUM accumulation.

Performance structure (HW-validated):
  * The kernel is PE-sequencer dispatch-bound: the recurrence needs
    48 W_hh matmuls (+48 ldweights) per step -- the ISA minimum for a
    [1536x512]@[512x32] product -- and each PE instruction costs ~16-20ns
    to dispatch.  All other engines are kept off that critical resource.
  * The input projections xi = x @ W_ih.T + b are computed as wide
    512/256-column matmuls in 16-timestep blocks (layer 1 in half-blocks so
    it can start after only half of layer 0\'s block), software-pipelined
    against the recurrences, and drained on the ACT engine with the bias
    folded in.
  * The r gate lives in its own PSUM bank so its sigmoid waits only on the
    16 r matmuls; n-row matmuls issue before z so the n-path is ready when
    sigmoid(r) completes.  Gate elementwise ops are bf16 (2x DVE mode); the
    z-path products z*h and 1-z run on the otherwise-idle GpSimd engine
    (moving them to DVE measures ~0.6ms/rep slower -- DVE queue congestion).
  * The hidden state is written bf16 directly into the sequence tile by the
    last gate op -- no per-step copies; the fc head consumes it directly.
  * Water-filling runs 12 rounds (provably >= 11 suffice: sum(w)=1, cap
    0.1 -> at most 10 elements ever cap) with the per-round sum fused into
    the clip via accum_out.
"""

import os
import numpy as np
import ml_dtypes

import concourse.bass as bass
import concourse.mybir as mybir
import concourse.tile as tile
from concourse.bass_utils import run_bass_kernel_spmd

BF = ml_dtypes.bfloat16
F32 = mybir.dt.float32
BF16 = mybir.dt.bfloat16
OP = mybir.AluOpType
AF = mybir.ActivationFunctionType

B, T, N, H = 256, 128, 512, 512
NCORES = 8
BS = B // NCORES            # 32 samples per core
KC = H // 128               # 4 contraction chunks
MC = 3 * H // 128           # 12 gate-row chunks (r: 0-3, z: 4-7, n: 8-11)
TB = 16                     # timesteps per pipeline block
NBLK = int(os.environ.get("GRU_NBLK", T // TB))  # dev knob; 8 = full
UB = 0.1
NITER_WF = 12               # water-filling rounds (>= 11 provably converged:
                            # sum(w)=1, UB=0.1 -> at most 10 elements can cap,
                            # and each non-final round caps at least one new)
REPS = int(os.environ.get("GRU_REPS", "1"))  # repeat whole kernel (timing only)


def _split_sync(nc, max_waits=1, max_updates=1):
    """This container's walrus accepts only one sync wait per instruction.
    Move extra waits onto same-engine NoOps placed just before; move extra
    updates of compute instructions onto NoOps just after (engines complete
    in order).  DMA instructions must keep a single update (async completion)
    so assert they do."""
    for f in nc.m.functions:
        for bb in f.blocks:
            out = []
            changed = False
            for inst in bb.instructions:
                si = getattr(inst, "sync_info", None)
                pre, post = [], []
                if si is not None and si.on_wait and len(si.on_wait) > max_waits:
                    waits = list(si.on_wait)
                    extra, keep = waits[:-max_waits], waits[-max_waits:]
                    i = 0
                    while extra:
                        chunk, extra = extra[:max_waits], extra[max_waits:]
                        nop = mybir.InstNoOp(name=f"{inst.name}-ws{i}", ins=[], outs=[])
                        nop.engine = inst.engine
                        nop.sync_info = mybir.SyncInfo(on_wait=chunk, on_update=[])
                        pre.append(nop)
                        i += 1
                    inst.sync_info = mybir.SyncInfo(
                        on_wait=keep, on_update=list(si.on_update)
                    )
                    si = inst.sync_info
                if si is not None and si.on_update and len(si.on_update) > max_updates:
                    assert not isinstance(inst, mybir.InstTensorCopy) and (
                        "DMA" not in type(inst).__name__
                    ), f"multi-update DMA {inst.name} cannot be split"
                    ups = list(si.on_update)
                    keep_u, extra_u = ups[:max_updates], ups[max_updates:]
                    i = 0
                    while extra_u:
                        chunk, extra_u = extra_u[:max_updates], extra_u[max_updates:]
                        nop = mybir.InstNoOp(name=f"{inst.name}-us{i}", ins=[], outs=[])
                        nop.engine = inst.engine
                        nop.sync_info = mybir.SyncInfo(on_wait=[], on_update=chunk)
                        post.append(nop)
                        i += 1
                    inst.sync_info = mybir.SyncInfo(
                        on_wait=list(si.on_wait), on_update=keep_u
                    )
                if pre or post:
                    changed = True
                out.extend(pre)
                out.append(inst)
                out.extend(post)
            if changed:
                bb.instructions = out
    return nc


def _build():
    nc = bass.Bass()
    dp = nc.declare_dram_parameter
    xts_e = dp("xts", [NBLK, 128, KC, TB, BS], BF16, isOutput=False)
    wih0_e = dp("wih0", [128, KC, 3 * H], BF16, isOutput=False)
    whh0_e = dp("whh0", [128, KC, 3 * H], BF16, isOutput=False)
    wih1_e = dp("wih1", [128, KC, 3 * H], BF16, isOutput=False)
    whh1_e = dp("whh1", [128, KC, 3 * H], BF16, isOutput=False)
    fcw_e = dp("fcw", [128, KC, N], BF16, isOutput=False)
    bias0_e = dp("bias0", [128, MC], F32, isOutput=False)  # drain bias, layer 0
    bias1_e = dp("bias1", [128, MC], F32, isOutput=False)
    bhn0_e = dp("bhn0", [128, KC, BS], F32, isOutput=False)  # b_hh0 n bcast
    bhn1_e = dp("bhn1", [128, KC, BS], F32, isOutput=False)
    fcb_e = dp("fcb", [BS, N], F32, isOutput=False)  # fc bias replicated per row
    y_e = dp("y", [BS, N], F32, isOutput=True)

    with tile.TileContext(nc) as tc:
        with (
            tc.tile_pool(name="wpool", bufs=1) as wp,
            tc.tile_pool(name="xpool", bufs=3) as xp,
            tc.tile_pool(name="hseq", bufs=3) as hp,
            tc.tile_pool(name="xipool", bufs=4) as xip,
            tc.tile_pool(name="state", bufs=2) as sp,
            tc.tile_pool(name="gates", bufs=3) as gp,
            tc.tile_pool(name="head", bufs=2) as hd,
            tc.tile_pool(name="ps_xi", bufs=2, space="PSUM") as ps_xi,
            tc.tile_pool(name="ps_r", bufs=2, space="PSUM") as ps_r,
            tc.tile_pool(name="ps_zn", bufs=2, space="PSUM") as ps_zn,
            tc.tile_pool(name="ps_fc", bufs=1, space="PSUM") as ps_fc,
        ):
            # ---- resident weights/constants -------------------------------
            wih0 = wp.tile([128, KC, 3 * H], BF16)
            nc.sync.dma_start(wih0[:], wih0_e[:])
            whh0 = wp.tile([128, KC, 3 * H], BF16)
            nc.sync.dma_start(whh0[:], whh0_e[:])
            wih1 = wp.tile([128, KC, 3 * H], BF16)
            nc.sync.dma_start(wih1[:], wih1_e[:])
            whh1 = wp.tile([128, KC, 3 * H], BF16)
            nc.sync.dma_start(whh1[:], whh1_e[:])
            fcw = wp.tile([128, KC, N], BF16)
            nc.sync.dma_start(fcw[:], fcw_e[:])
            bias0 = wp.tile([128, MC], F32)
            nc.sync.dma_start(bias0[:], bias0_e[:])
            bias1 = wp.tile([128, MC], F32)
            nc.sync.dma_start(bias1[:], bias1_e[:])
            bhn0 = wp.tile([128, KC, BS], F32)
            nc.sync.dma_start(bhn0[:], bhn0_e[:])
            bhn1 = wp.tile([128, KC, BS], F32)
            nc.sync.dma_start(bhn1[:], bhn1_e[:])
            fcb = wp.tile([BS, N], F32)
            nc.sync.dma_start(fcb[:], fcb_e[:])
            zrhs = wp.tile([128, KC, BS], BF16)
            nc.vector.memset(zrhs[:], 0.0)

            for rep in range(REPS):
                xt_tiles = [None] * NBLK
                hs_tiles = [None] * NBLK
                xi_tiles = [[None] * NBLK, [None] * NBLK]
                h1b_box = [zrhs]

                def load_x_block(c):
                    xt = xp.tile([128, KC, TB, BS], BF16, tag="xt")
                    nc.sync.dma_start(xt[:], xts_e[c])
                    xt_tiles[c] = xt

                def queue_xi_block(layer, c, tasks, half=None):
                    """Queue the 12 m-chunk matmuls of xi[layer][c].

                    half=0/1 (layer 1 only): compute timesteps [0,TB/2) or
                    [TB/2,TB) so layer 1 can start after only half of layer
                    0's block is done."""
                    if half in (None, 0):
                        xi = xip.tile(
                            [128, TB, MC, BS], BF16, tag="xi",
                            name=f"xi{layer}_{c}_{rep}",
                        )
                        xi_tiles[layer][c] = xi
                    else:
                        xi = xi_tiles[layer][c]
                    w = wih0 if layer == 0 else wih1
                    bias = bias0 if layer == 0 else bias1
                    rhs = xt_tiles[c] if layer == 0 else hs_tiles[c]
                    if half is None:
                        t0_, t1_ = 0, TB
                    else:
                        t0_, t1_ = half * (TB // 2), (half + 1) * (TB // 2)
                    nt = t1_ - t0_

                    acc_box = {}

                    def mm_piece(m, k):
                        if k == 0:
                            acc_box[m] = ps_xi.tile(
                                [128, TB * BS], F32, tag="psxi",
                                name=f"psxi{layer}_{c}_{m}_{half}_{rep}",
                            )
                        nc.tensor.matmul(
                            acc_box[m][:, : nt * BS],
                            w[:, k, 128 * m : 128 * (m + 1)],
                            rhs[:, k, t0_:t1_, :],
                            start=(k == 0),
                            stop=(k == KC - 1),
                        )
                        if k == KC - 1:
                            acc = acc_box.pop(m)
                            nc.scalar.activation(
                                xi[:, t0_:t1_, m, :],
                                acc[:, : nt * BS].rearrange(
                                    "p (t b) -> p t b", b=BS
                                ),
                                AF.Identity,
                                bias=bias[:, m : m + 1],
                            )

                    for m in range(MC):
                        for k in range(KC):
                            tasks.append(
                                ((layer, c, half), lambda m=m, k=k: mm_piece(m, k))
                            )

                def rec_step(layer, c, ti, t):
                    whh = whh0 if layer == 0 else whh1
                    bhn = bhn0 if layer == 0 else bhn1
                    xi = xi_tiles[layer][c]
                    if layer == 0:
                        if t == 0:
                            rsl = lambda k: zrhs[:, k, :]
                            hprev = None
                        elif ti == 0:
                            prev = hs_tiles[c - 1]
                            rsl = lambda k: prev[:, k, TB - 1, :]
                            hprev = prev[:, :, TB - 1, :]
                        else:
                            cur = hs_tiles[c]
                            rsl = lambda k: cur[:, k, ti - 1, :]
                            hprev = cur[:, :, ti - 1, :]
                    else:
                        hb = h1b_box[0]
                        rsl = lambda k: hb[:, k, :]
                        hprev = hb[:, :, :] if t > 0 else None

                    # ---- PE: W_hh gate matmuls into PSUM ------------------
                    gr = ps_r.tile([128, KC, BS], F32, tag="gr")
                    for m in range(4):  # r rows
                        for k in range(KC):
                            nc.tensor.matmul(
                                gr[:, m, :],
                                whh[:, k, 128 * m : 128 * (m + 1)],
                                rsl(k),
                                start=(k == 0),
                                stop=(k == KC - 1),
                            )
                    gzn = ps_zn.tile([128, 2 * KC, BS], F32, tag="gzn")
                    # n rows first (t1 needs them right after sigmoid(r))
                    for m in range(8, 12):
                        for k in range(KC):
                            nc.tensor.matmul(
                                gzn[:, m - 4, :],
                                whh[:, k, 128 * m : 128 * (m + 1)],
                                rsl(k),
                                start=(k == 0),
                                stop=(k == KC - 1),
                            )
                    for m in range(4, 8):  # z rows
                        for k in range(KC):
                            nc.tensor.matmul(
                                gzn[:, m - 4, :],
                                whh[:, k, 128 * m : 128 * (m + 1)],
                                rsl(k),
                                start=(k == 0),
                                stop=(k == KC - 1),
                            )

                    # ---- gates (bf16 elementwise, [128, KC, BS]) ----------
                    prr = gp.tile([128, KC, BS], F32, tag="prr")
                    nc.vector.tensor_add(prr[:], gr[:], xi[:, ti, 0:4, :])
                    r = gp.tile([128, KC, BS], BF16, tag="r")
                    nc.scalar.activation(r[:], prr[:], AF.Sigmoid)
                    prz = gp.tile([128, KC, BS], F32, tag="prz")
                    nc.vector.tensor_add(prz[:], gzn[:, 0:4, :], xi[:, ti, 4:8, :])
                    z = gp.tile([128, KC, BS], BF16, tag="z")
                    nc.scalar.activation(z[:], prz[:], AF.Sigmoid)
                    hn = gp.tile([128, KC, BS], BF16, tag="hn")
                    nc.vector.tensor_add(hn[:], gzn[:, 4:8, :], bhn[:])
                    zb = gp.tile([128, KC, BS], BF16, tag="zb")
                    nc.gpsimd.tensor_scalar(zb[:], z[:], -1.0, 1.0, OP.mult, OP.add)
                    if hprev is not None:
                        zh = gp.tile([128, KC, BS], BF16, tag="zh")
                        nc.gpsimd.tensor_mul(zh[:], z[:], hprev)
                    t1 = gp.tile([128, KC, BS], BF16, tag="t1")
                    nc.vector.tensor_mul(t1[:], hn[:], r[:])
                    pn = gp.tile([128, KC, BS], BF16, tag="pn")
                    nc.vector.tensor_add(pn[:], t1[:], xi[:, ti, 8:12, :])
                    n_t = gp.tile([128, KC, BS], BF16, tag="n_t")
                    nc.scalar.activation(n_t[:], pn[:], AF.Tanh)

                    if layer == 0:
                        target = hs_tiles[c][:, :, ti, :]
                    else:
                        h1b = sp.tile([128, KC, BS], BF16, tag="h1b")
                        target = h1b[:]
                        h1b_box[0] = h1b
                    if hprev is None:
                        nc.vector.tensor_mul(target, n_t[:], zb[:])
                    else:
                        m1 = gp.tile([128, KC, BS], BF16, tag="m1")
                        nc.vector.tensor_mul(m1[:], n_t[:], zb[:])
                        nc.vector.tensor_add(target, m1[:], zh[:])

                # ---- step-interleaved pipelined schedule ------------------
                LAG = TB // 2 + 4
                tasks = []

                def force_tasks(key):
                    rest = [tk for tk in tasks if tk[0] == key]
                    tasks[:] = [tk for tk in tasks if tk[0] != key]
                    for _, fn in rest:
                        fn()

                load_x_block(0)
                queue_xi_block(0, 0, tasks)
                while tasks:  # xi0 block 0 fully before the loop
                    tasks.pop(0)[1]()
                TRUN = NBLK * TB
                for i in range(TRUN + LAG):
                    if i < TRUN:
                        c, ti = divmod(i, TB)
                        if ti == 0:
                            if c + 1 < NBLK:
                                load_x_block(c + 1)
                                queue_xi_block(0, c + 1, tasks)
                            hs_tiles[c] = hp.tile(
                                [128, KC, TB, BS], BF16, tag="hs",
                                name=f"hs{c}_{rep}",
                            )
                        rec_step(0, c, ti, i)
                        if ti == TB // 2 - 1:
                            queue_xi_block(1, c, tasks, half=0)
                        elif ti == TB - 1:
                            queue_xi_block(1, c, tasks, half=1)
                    j = i - LAG
                    if 0 <= j < TRUN:
                        jc, jti = divmod(j, TB)
                        if jti == 0:
                            force_tasks((1, jc, 0))
                        elif jti == TB // 2:
                            force_tasks((1, jc, 1))
                        rec_step(1, jc, jti, j)
                    for _ in range(8):
                        if tasks:
                            tasks.pop(0)[1]()

                # ---- head: fc + silu + softmax + water-filling -----------
                h1f = h1b_box[0]
                lp = ps_fc.tile([BS, N], F32)
                for k in range(KC):
                    nc.tensor.matmul(
                        lp[:], h1f[:, k, :], fcw[:, k, :],
                        start=(k == 0), stop=(k == KC - 1),
                    )
                lg = hd.tile([BS, N], F32, tag="lg")
                nc.vector.tensor_add(lg[:], lp[:], fcb[:])
                sl = hd.tile([BS, N], F32, tag="sl")
                nc.scalar.activation(sl[:], lg[:], AF.Silu)
                mx = hd.tile([BS, 1], F32, tag="mx")
                nc.vector.reduce_max(mx[:], sl[:], axis=mybir.AxisListType.X)
                nmx = hd.tile([BS, 1], F32, tag="nmx")
                nc.vector.tensor_scalar_mul(nmx[:], mx[:], -1.0)
                ex = hd.tile([BS, N], F32, tag="ex")
                nc.scalar.activation(ex[:], sl[:], AF.Exp, bias=nmx[:])
                se = hd.tile([BS, 1], F32, tag="se")
                nc.vector.reduce_sum(se[:], ex[:], axis=mybir.AxisListType.X)
                rs = hd.tile([BS, 1], F32, tag="rs")
                nc.vector.reciprocal(rs[:], se[:])
                w = hd.tile([BS, N], F32, tag="w")
                nc.vector.tensor_scalar_mul(w[:], ex[:], rs[:])
                t0 = hd.tile([BS, 1], F32, tag="t0")
                nc.vector.reduce_sum(t0[:], w[:], axis=mybir.AxisListType.X)
                wc = hd.tile([BS, N], F32, tag="w")
                swc = hd.tile([BS, 1], F32, tag="swc")
                nc.vector.tensor_scalar(
                    wc[:], w[:], UB, None, OP.min, OP.add, accum_out=swc[:]
                )
                for _ in range(NITER_WF):
                    noms = hd.tile([BS, N], F32, tag="noms")
                    s_n = hd.tile([BS, 1], F32, tag="s_n")
                    nc.vector.scalar_tensor_tensor(
                        noms[:], wc[:], UB, wc[:], OP.is_lt, OP.mult,
                        accum_out=s_n[:],
                    )
                    lft = hd.tile([BS, 1], F32, tag="lft")
                    nc.vector.tensor_scalar(
                        lft[:], swc[:], -1.0, t0[:], OP.mult, OP.add
                    )
                    rsn = hd.tile([BS, 1], F32, tag="rsn")
                    nc.vector.reciprocal(rsn[:], s_n[:])
                    gg = hd.tile([BS, 1], F32, tag="gg")
                    nc.vector.tensor_mul(gg[:], lft[:], rsn[:])
                    w2 = hd.tile([BS, N], F32, tag="noms2")
                    nc.vector.scalar_tensor_tensor(
                        w2[:], noms[:], gg[:], wc[:], OP.mult, OP.add
                    )
                    wc = hd.tile([BS, N], F32, tag="w")
                    swc = hd.tile([BS, 1], F32, tag="swc")
                    nc.vector.tensor_scalar(
                        wc[:], w2[:], UB, None, OP.min, OP.add, accum_out=swc[:]
                    )
                nc.sync.dma_start(y_e[:], wc[:])

    _split_sync(nc)
    return nc


def _prep_inputs(x, W_ih0, W_hh0, b_ih0, b_hh0, W_ih1, W_hh1, b_ih1, b_hh1,
                 fc_w, fc_b):
    """Host-side layout prep: transpose/shard/cast; returns per-core in_maps."""
    def wT(w):  # [3H, in] -> [128, KC, 3H] bf16 (lhsT chunks)
        wt = np.ascontiguousarray(w.T.reshape(KC, 128, 3 * H).transpose(1, 0, 2))
        return wt.astype(BF)

    def bias_comb(b_ih, b_hh):  # rz rows: both; n rows: b_ih only
        b = b_ih.astype(np.float64) + np.concatenate(
            [b_hh[: 2 * H], np.zeros(H)]
        )
        return np.ascontiguousarray(
            b.astype(np.float32).reshape(MC, 128).T
        )  # [128, MC]

    def bhn(b_hh):  # n-part [H] -> [128, KC, BS] broadcast over batch
        v = b_hh[2 * H :].astype(np.float32).reshape(KC, 128).T  # [128, KC]
        return np.ascontiguousarray(
            np.broadcast_to(v[:, :, None], (128, KC, BS))
        )

    fcw = np.ascontiguousarray(
        fc_w.T.reshape(KC, 128, N).transpose(1, 0, 2).astype(BF)
    )
    fcb = np.ascontiguousarray(
        np.broadcast_to(fc_b[None, :], (BS, N)).astype(np.float32)
    )

    shared = {
        "wih0": wT(W_ih0), "whh0": wT(W_hh0),
        "wih1": wT(W_ih1), "whh1": wT(W_hh1),
        "fcw": fcw,
        "bias0": bias_comb(b_ih0, b_hh0), "bias1": bias_comb(b_ih1, b_hh1),
        "bhn0": bhn(b_hh0), "bhn1": bhn(b_hh1),
        "fcb": fcb,
    }
    in_maps = []
    for core in range(NCORES):
        xb = x[core * BS : (core + 1) * BS].astype(BF)  # [BS, T, N]
        # [N, T, BS] -> [KC, 128, NBLK, TB, BS] -> [NBLK, 128, KC, TB, BS]
        xt = xb.transpose(2, 1, 0).reshape(KC, 128, T // TB, TB, BS)
        xt = np.ascontiguousarray(xt.transpose(2, 1, 0, 3, 4))[:NBLK]
        m = dict(shared)
        m["xts"] = np.ascontiguousarray(xt)
        in_maps.append(m)
    return in_maps


_NC_CACHE = {}


def _get_nc():
    if "nc" not in _NC_CACHE:
        _NC_CACHE["nc"] = _build()
    return _NC_CACHE["nc"]


def kernel(**inputs):
    nc = _get_nc()
    in_maps = _prep_inputs(**{k: np.asarray(v) for k, v in inputs.items()})
    res = run_bass_kernel_spmd(nc, in_maps, list(range(NCORES)))
    return np.concatenate([res.results[i]["y"] for i in range(NCORES)], axis=0)


if __name__ == "__main__":
    rng = np.random.default_rng(0)
    ins = {
        "x": rng.standard_normal((B, T, N), dtype=np.float32),
        "W_ih0": rng.standard_normal((3 * H, N), dtype=np.float32) * 0.04,
        "W_hh0": rng.standard_normal((3 * H, H), dtype=np.float32) * 0.04,
        "b_ih0": rng.standard_normal(3 * H).astype(np.float32) * 0.04,
        "b_hh0": rng.standard_normal(3 * H).astype(np.float32) * 0.04,
        "W_ih1": rng.standard_normal((3 * H, H), dtype=np.float32) * 0.04,
        "W_hh1": rng.standard_normal((3 * H, H), dtype=np.float32) * 0.04,
        "b_ih1": rng.standard_normal(3 * H).astype(np.float32) * 0.04,
        "b_hh1": rng.standard_normal(3 * H).astype(np.float32) * 0.04,
        "fc_w": rng.standard_normal((N, H), dtype=np.float32) * 0.04,
        "fc_b": rng.standard_normal(N).astype(np.float32) * 0.04,
    }
    out = kernel(**ins)
    print("out", out.shape, out.dtype, out.sum())


# revision 28
# speedup vs baseline: 108.1791x; 1.0932x over previous
"""Trainium2 Bass kernel for nn_GRU_77025943486969.

Reference computation (see problem spec):
  2-layer GRU (B=256, T=128, N=H=512)  ->  fc  ->  silu  ->  softmax
  ->  per-sample iterative water-filling (clip to [0, 0.1], redistribute).

Strategy: data-parallel over batch across 8 NeuronCores (32 samples/core).
Per core everything runs in a transposed layout: hidden dim on the 128
partitions (4 chunks of 128), batch in the free dimension (32), so that the
gate elementwise math uses all 128 DVE/ACT lanes.  Matmuls run in bf16 with
fp32 PSUM accumulation.

Performance structure (HW-validated):
  * The kernel is PE-sequencer dispatch-bound: the recurrence needs
    48 W_hh matmuls (+48 ldweights) per step -- the ISA minimum for a
    [1536x512]@[512x32] product -- and each PE instruction costs ~16-20ns
    to dispatch.  All other engines are kept off that critical resource.
  * The input projections xi = x @ W_ih.T + b are computed as wide
    512/256-column matmuls in 16-timestep blocks (layer 1 in half-blocks so
    it can start after only half of layer 0\'s block), software-pipelined
    against the recurrences, and drained on the ACT engine with the bias
    folded in.
  * The r gate lives in its own PSUM bank so its sigmoid waits only on the
    16 r matmuls; n-row matmuls issue before z so the n-path is ready when
    sigmoid(r) completes.  Gate elementwise ops are bf16 (2x DVE mode); the
    z-path products z*h and 1-z run on the otherwise-idle GpSimd engine
    (moving them to DVE measures ~0.6ms/rep slower -- DVE queue congestion).
  * The hidden state is written bf16 directly into the sequence tile by the
    last gate op -- no per-step copies; the fc head consumes it directly.
  * Water-filling runs 12 rounds (provably >= 11 suffice: sum(w)=1, cap
    0.1 -> at most 10 elements ever cap) with the per-round sum fused into
    the clip via accum_out; round tensors are bf16 (2x DVE) with all
    cancellation-sensitive per-sample scalars kept fp32.
"""

import os
import numpy as np
import ml_dtypes

import concourse.bass as bass
import concourse.mybir as mybir
import concourse.tile as tile
from concourse.bass_utils import run_bass_kernel_spmd

BF = ml_dtypes.bfloat16
F32 = mybir.dt.float32
BF16 = mybir.dt.bfloat16
OP = mybir.AluOpType
AF = mybir.ActivationFunctionType

B, T, N, H = 256, 128, 512, 512
NCORES = 8
BS = B // NCORES            # 32 samples per core
KC = H // 128               # 4 contraction chunks
MC = 3 * H // 128           # 12 gate-row chunks (r: 0-3, z: 4-7, n: 8-11)
TB = 16                     # timesteps per pipeline block
NBLK = int(os.environ.get("GRU_NBLK", T // TB))  # dev knob; 8 = full
UB = 0.1
NITER_WF = 12               # water-filling rounds (>= 11 provably converged:
                            # sum(w)=1, UB=0.1 -> at most 10 elements can cap,
                            # and each non-final round caps at least one new)
REPS = int(os.environ.get("GRU_REPS", "1"))  # repeat whole kernel (timing only)


def _split_sync(nc, max_waits=1, max_updates=1):
    """This container's walrus accepts only one sync wait per instruction.
    Move extra waits onto same-engine NoOps placed just before; move extra
    updates of compute instructions onto NoOps just after (engines complete
    in order).  DMA instructions must keep a single update (async completion)
    so assert they do."""
    for f in nc.m.functions:
        for bb in f.blocks:
            out = []
            changed = False
            for inst in bb.instructions:
                si = getattr(inst, "sync_info", None)
                pre, post = [], []
                if si is not None and si.on_wait and len(si.on_wait) > max_waits:
                    waits = list(si.on_wait)
                    extra, keep = waits[:-max_waits], waits[-max_waits:]
                    i = 0
                    while extra:
                        chunk, extra = extra[:max_waits], extra[max_waits:]
                        nop = mybir.InstNoOp(name=f"{inst.name}-ws{i}", ins=[], outs=[])
                        nop.engine = inst.engine
                        nop.sync_info = mybir.SyncInfo(on_wait=chunk, on_update=[])
                        pre.append(nop)
                        i += 1
                    inst.sync_info = mybir.SyncInfo(
                        on_wait=keep, on_update=list(si.on_update)
                    )
                    si = inst.sync_info
                if si is not None and si.on_update and len(si.on_update) > max_updates:
                    assert not isinstance(inst, mybir.InstTensorCopy) and (
                        "DMA" not in type(inst).__name__
                    ), f"multi-update DMA {inst.name} cannot be split"
                    ups = list(si.on_update)
                    keep_u, extra_u = ups[:max_updates], ups[max_updates:]
                    i = 0
                    while extra_u:
                        chunk, extra_u = extra_u[:max_updates], extra_u[max_updates:]
                        nop = mybir.InstNoOp(name=f"{inst.name}-us{i}", ins=[], outs=[])
                        nop.engine = inst.engine
                        nop.sync_info = mybir.SyncInfo(on_wait=[], on_update=chunk)
                        post.append(nop)
                        i += 1
                    inst.sync_info = mybir.SyncInfo(
                        on_wait=list(si.on_wait), on_update=keep_u
                    )
                if pre or post:
                    changed = True
                out.extend(pre)
                out.append(inst)
                out.extend(post)
            if changed:
                bb.instructions = out
    return nc


def _build():
    nc = bass.Bass()
    dp = nc.declare_dram_parameter
    xts_e = dp("xts", [NBLK, 128, KC, TB, BS], BF16, isOutput=False)
    wih0_e = dp("wih0", [128, KC, 3 * H], BF16, isOutput=False)
    whh0_e = dp("whh0", [128, KC, 3 * H], BF16, isOutput=False)
    wih1_e = dp("wih1", [128, KC, 3 * H], BF16, isOutput=False)
    whh1_e = dp("whh1", [128, KC, 3 * H], BF16, isOutput=False)
    fcw_e = dp("fcw", [128, KC, N], BF16, isOutput=False)
    bias0_e = dp("bias0", [128, MC], F32, isOutput=False)  # drain bias, layer 0
    bias1_e = dp("bias1", [128, MC], F32, isOutput=False)
    bhn0_e = dp("bhn0", [128, KC, BS], F32, isOutput=False)  # b_hh0 n bcast
    bhn1_e = dp("bhn1", [128, KC, BS], F32, isOutput=False)
    fcb_e = dp("fcb", [BS, N], F32, isOutput=False)  # fc bias replicated per row
    y_e = dp("y", [BS, N], F32, isOutput=True)

    with tile.TileContext(nc) as tc:
        with (
            tc.tile_pool(name="wpool", bufs=1) as wp,
            tc.tile_pool(name="xpool", bufs=3) as xp,
            tc.tile_pool(name="hseq", bufs=3) as hp,
            tc.tile_pool(name="xipool", bufs=4) as xip,
            tc.tile_pool(name="state", bufs=2) as sp,
            tc.tile_pool(name="gates", bufs=3) as gp,
            tc.tile_pool(name="head", bufs=2) as hd,
            tc.tile_pool(name="ps_xi", bufs=2, space="PSUM") as ps_xi,
            tc.tile_pool(name="ps_r", bufs=2, space="PSUM") as ps_r,
            tc.tile_pool(name="ps_zn", bufs=2, space="PSUM") as ps_zn,
            tc.tile_pool(name="ps_fc", bufs=1, space="PSUM") as ps_fc,
        ):
            # ---- resident weights/constants -------------------------------
            wih0 = wp.tile([128, KC, 3 * H], BF16)
            nc.sync.dma_start(wih0[:], wih0_e[:])
            whh0 = wp.tile([128, KC, 3 * H], BF16)
            nc.sync.dma_start(whh0[:], whh0_e[:])
            wih1 = wp.tile([128, KC, 3 * H], BF16)
            nc.sync.dma_start(wih1[:], wih1_e[:])
            whh1 = wp.tile([128, KC, 3 * H], BF16)
            nc.sync.dma_start(whh1[:], whh1_e[:])
            fcw = wp.tile([128, KC, N], BF16)
            nc.sync.dma_start(fcw[:], fcw_e[:])
            bias0 = wp.tile([128, MC], F32)
            nc.sync.dma_start(bias0[:], bias0_e[:])
            bias1 = wp.tile([128, MC], F32)
            nc.sync.dma_start(bias1[:], bias1_e[:])
            bhn0 = wp.tile([128, KC, BS], F32)
            nc.sync.dma_start(bhn0[:], bhn0_e[:])
            bhn1 = wp.tile([128, KC, BS], F32)
            nc.sync.dma_start(bhn1[:], bhn1_e[:])
            fcb = wp.tile([BS, N], F32)
            nc.sync.dma_start(fcb[:], fcb_e[:])
            zrhs = wp.tile([128, KC, BS], BF16)
            nc.vector.memset(zrhs[:], 0.0)

            for rep in range(REPS):
                xt_tiles = [None] * NBLK
                hs_tiles = [None] * NBLK
                xi_tiles = [[None] * NBLK, [None] * NBLK]
                h1b_box = [zrhs]

                def load_x_block(c):
                    xt = xp.tile([128, KC, TB, BS], BF16, tag="xt")
                    nc.sync.dma_start(xt[:], xts_e[c])
                    xt_tiles[c] = xt

                def queue_xi_block(layer, c, tasks, half=None):
                    """Queue the 12 m-chunk matmuls of xi[layer][c].

                    half=0/1 (layer 1 only): compute timesteps [0,TB/2) or
                    [TB/2,TB) so layer 1 can start after only half of layer
                    0's block is done."""
                    if half in (None, 0):
                        xi = xip.tile(
                            [128, TB, MC, BS], BF16, tag="xi",
                            name=f"xi{layer}_{c}_{rep}",
                        )
                        xi_tiles[layer][c] = xi
                    else:
                        xi = xi_tiles[layer][c]
                    w = wih0 if layer == 0 else wih1
                    bias = bias0 if layer == 0 else bias1
                    rhs = xt_tiles[c] if layer == 0 else hs_tiles[c]
                    if half is None:
                        t0_, t1_ = 0, TB
                    else:
                        t0_, t1_ = half * (TB // 2), (half + 1) * (TB // 2)
                    nt = t1_ - t0_

                    acc_box = {}

                    def mm_piece(m, k):
                        if k == 0:
                            acc_box[m] = ps_xi.tile(
                                [128, TB * BS], F32, tag="psxi",
                                name=f"psxi{layer}_{c}_{m}_{half}_{rep}",
                            )
                        nc.tensor.matmul(
                            acc_box[m][:, : nt * BS],
                            w[:, k, 128 * m : 128 * (m + 1)],
                            rhs[:, k, t0_:t1_, :],
                            start=(k == 0),
                            stop=(k == KC - 1),
                        )
                        if k == KC - 1:
                            acc = acc_box.pop(m)
                            nc.scalar.activation(
                                xi[:, t0_:t1_, m, :],
                                acc[:, : nt * BS].rearrange(
                                    "p (t b) -> p t b", b=BS
                                ),
                                AF.Identity,
                                bias=bias[:, m : m + 1],
                            )

                    for m in range(MC):
                        for k in range(KC):
                            tasks.append(
                                ((layer, c, half), lambda m=m, k=k: mm_piece(m, k))
                            )

                def rec_step(layer, c, ti, t):
                    whh = whh0 if layer == 0 else whh1
                    bhn = bhn0 if layer == 0 else bhn1
                    xi = xi_tiles[layer][c]
                    if layer == 0:
                        if t == 0:
                            rsl = lambda k: zrhs[:, k, :]
                            hprev = None
                        elif ti == 0:
                            prev = hs_tiles[c - 1]
                            rsl = lambda k: prev[:, k, TB - 1, :]
                            hprev = prev[:, :, TB - 1, :]
                        else:
                            cur = hs_tiles[c]
                            rsl = lambda k: cur[:, k, ti - 1, :]
                            hprev = cur[:, :, ti - 1, :]
                    else:
                        hb = h1b_box[0]
                        rsl = lambda k: hb[:, k, :]
                        hprev = hb[:, :, :] if t > 0 else None

                    # ---- PE: W_hh gate matmuls into PSUM ------------------
                    gr = ps_r.tile([128, KC, BS], F32, tag="gr")
                    for m in range(4):  # r rows
                        for k in range(KC):
                            nc.tensor.matmul(
                                gr[:, m, :],
                                whh[:, k, 128 * m : 128 * (m + 1)],
                                rsl(k),
                                start=(k == 0),
                                stop=(k == KC - 1),
                            )
                    gzn = ps_zn.tile([128, 2 * KC, BS], F32, tag="gzn")
                    # n rows first (t1 needs them right after sigmoid(r))
                    for m in range(8, 12):
                        for k in range(KC):
                            nc.tensor.matmul(
                                gzn[:, m - 4, :],
                                whh[:, k, 128 * m : 128 * (m + 1)],
                                rsl(k),
                                start=(k == 0),
                                stop=(k == KC - 1),
                            )
                    for m in range(4, 8):  # z rows
                        for k in range(KC):
                            nc.tensor.matmul(
                                gzn[:, m - 4, :],
                                whh[:, k, 128 * m : 128 * (m + 1)],
                                rsl(k),
                                start=(k == 0),
                                stop=(k == KC - 1),
                            )

                    # ---- gates (bf16 elementwise, [128, KC, BS]) ----------
                    prr = gp.tile([128, KC, BS], F32, tag="prr")
                    nc.vector.tensor_add(prr[:], gr[:], xi[:, ti, 0:4, :])
                    r = gp.tile([128, KC, BS], BF16, tag="r")
                    nc.scalar.activation(r[:], prr[:], AF.Sigmoid)
                    prz = gp.tile([128, KC, BS], F32, tag="prz")
                    nc.vector.tensor_add(prz[:], gzn[:, 0:4, :], xi[:, ti, 4:8, :])
                    z = gp.tile([128, KC, BS], BF16, tag="z")
                    nc.scalar.activation(z[:], prz[:], AF.Sigmoid)
                    hn = gp.tile([128, KC, BS], BF16, tag="hn")
                    nc.vector.tensor_add(hn[:], gzn[:, 4:8, :], bhn[:])
                    zb = gp.tile([128, KC, BS], BF16, tag="zb")
                    nc.gpsimd.tensor_scalar(zb[:], z[:], -1.0, 1.0, OP.mult, OP.add)
                    if hprev is not None:
                        zh = gp.tile([128, KC, BS], BF16, tag="zh")
                        nc.gpsimd.tensor_mul(zh[:], z[:], hprev)
                    t1 = gp.tile([128, KC, BS], BF16, tag="t1")
                    nc.vector.tensor_mul(t1[:], hn[:], r[:])
                    pn = gp.tile([128, KC, BS], BF16, tag="pn")
                    nc.vector.tensor_add(pn[:], t1[:], xi[:, ti, 8:12, :])
                    n_t = gp.tile([128, KC, BS], BF16, tag="n_t")
                    nc.scalar.activation(n_t[:], pn[:], AF.Tanh)

                    if layer == 0:
                        target = hs_tiles[c][:, :, ti, :]
                    else:
                        h1b = sp.tile([128, KC, BS], BF16, tag="h1b")
                        target = h1b[:]
                        h1b_box[0] = h1b
                    if hprev is None:
                        nc.vector.tensor_mul(target, n_t[:], zb[:])
                    else:
                        m1 = gp.tile([128, KC, BS], BF16, tag="m1")
                        nc.vector.tensor_mul(m1[:], n_t[:], zb[:])
                        nc.vector.tensor_add(target, m1[:], zh[:])

                # ---- step-interleaved pipelined schedule ------------------
                LAG = TB // 2 + 2
                tasks = []

                def force_tasks(key):
                    rest = [tk for tk in tasks if tk[0] == key]
                    tasks[:] = [tk for tk in tasks if tk[0] != key]
                    for _, fn in rest:
                        fn()

                load_x_block(0)
                queue_xi_block(0, 0, tasks)
                while tasks:  # xi0 block 0 fully before the loop
                    tasks.pop(0)[1]()
                TRUN = NBLK * TB
                for i in range(TRUN + LAG):
                    if i < TRUN:
                        c, ti = divmod(i, TB)
                        if ti == 0:
                            if c + 1 < NBLK:
                                load_x_block(c + 1)
                                queue_xi_block(0, c + 1, tasks)
                            hs_tiles[c] = hp.tile(
                                [128, KC, TB, BS], BF16, tag="hs",
                                name=f"hs{c}_{rep}",
                            )
                        rec_step(0, c, ti, i)
                        if ti == TB // 2 - 1:
                            queue_xi_block(1, c, tasks, half=0)
                        elif ti == TB - 1:
                            queue_xi_block(1, c, tasks, half=1)
                    j = i - LAG
                    if 0 <= j < TRUN:
                        jc, jti = divmod(j, TB)
                        if jti == 0:
                            force_tasks((1, jc, 0))
                        elif jti == TB // 2:
                            force_tasks((1, jc, 1))
                        rec_step(1, jc, jti, j)
                    for _ in range(8):
                        if tasks:
                            tasks.pop(0)[1]()

                # ---- head: fc + silu + softmax + water-filling -----------
                h1f = h1b_box[0]
                lp = ps_fc.tile([BS, N], F32)
                for k in range(KC):
                    nc.tensor.matmul(
                        lp[:], h1f[:, k, :], fcw[:, k, :],
                        start=(k == 0), stop=(k == KC - 1),
                    )
                lg = hd.tile([BS, N], F32, tag="lg")
                nc.vector.tensor_add(lg[:], lp[:], fcb[:])
                sl = hd.tile([BS, N], F32, tag="sl")
                nc.scalar.activation(sl[:], lg[:], AF.Silu)
                mx = hd.tile([BS, 1], F32, tag="mx")
                nc.vector.reduce_max(mx[:], sl[:], axis=mybir.AxisListType.X)
                nmx = hd.tile([BS, 1], F32, tag="nmx")
                nc.vector.tensor_scalar_mul(nmx[:], mx[:], -1.0)
                ex = hd.tile([BS, N], F32, tag="ex")
                nc.scalar.activation(ex[:], sl[:], AF.Exp, bias=nmx[:])
                se = hd.tile([BS, 1], F32, tag="se")
                nc.vector.reduce_sum(se[:], ex[:], axis=mybir.AxisListType.X)
                rs = hd.tile([BS, 1], F32, tag="rs")
                nc.vector.reciprocal(rs[:], se[:])
                w = hd.tile([BS, N], F32, tag="w")
                nc.vector.tensor_scalar_mul(w[:], ex[:], rs[:])
                t0 = hd.tile([BS, 1], F32, tag="t0")
                nc.vector.reduce_sum(t0[:], w[:], axis=mybir.AxisListType.X)
                wc = hd.tile([BS, N], BF16, tag="w")
                swc = hd.tile([BS, 1], F32, tag="swc")
                nc.vector.tensor_scalar(
                    wc[:], w[:], UB, None, OP.min, OP.add, accum_out=swc[:]
                )
                for it in range(NITER_WF):
                    last = it == NITER_WF - 1
                    noms = hd.tile([BS, N], BF16, tag="noms")
                    s_n = hd.tile([BS, 1], F32, tag="s_n")
                    nc.vector.scalar_tensor_tensor(
                        noms[:], wc[:], UB, wc[:], OP.is_lt, OP.mult,
                        accum_out=s_n[:],
                    )
                    lft = hd.tile([BS, 1], F32, tag="lft")
                    nc.vector.tensor_scalar(
                        lft[:], swc[:], -1.0, t0[:], OP.mult, OP.add
                    )
                    rsn = hd.tile([BS, 1], F32, tag="rsn")
                    nc.vector.reciprocal(rsn[:], s_n[:])
                    gg = hd.tile([BS, 1], F32, tag="gg")
                    nc.vector.tensor_mul(gg[:], lft[:], rsn[:])
                    w2 = hd.tile([BS, N], BF16, tag="noms2")
                    nc.vector.scalar_tensor_tensor(
                        w2[:], noms[:], gg[:], wc[:], OP.mult, OP.add
                    )
                    wc = hd.tile([BS, N], F32 if last else BF16, tag="wf" if last else "w")
                    swc = hd.tile([BS, 1], F32, tag="swc")
                    nc.vector.tensor_scalar(
                        wc[:], w2[:], UB, None, OP.min, OP.add, accum_out=swc[:]
                    )
                nc.sync.dma_start(y_e[:], wc[:])

    _split_sync(nc)
    return nc


def _prep_inputs(x, W_ih0, W_hh0, b_ih0, b_hh0, W_ih1, W_hh1, b_ih1, b_hh1,
                 fc_w, fc_b):
    """Host-side layout prep: transpose/shard/cast; returns per-core in_maps."""
    def wT(w):  # [3H, in] -> [128, KC, 3H] bf16 (lhsT chunks)
        wt = np.ascontiguousarray(w.T.reshape(KC, 128, 3 * H).transpose(1, 0, 2))
        return wt.astype(BF)

    def bias_comb(b_ih, b_hh):  # rz rows: both; n rows: b_ih only
        b = b_ih.astype(np.float64) + np.concatenate(
            [b_hh[: 2 * H], np.zeros(H)]
        )
        return np.ascontiguousarray(
            b.astype(np.float32).reshape(MC, 128).T
        )  # [128, MC]

    def bhn(b_hh):  # n-part [H] -> [128, KC, BS] broadcast over batch
        v = b_hh[2 * H :].astype(np.float32).reshape(KC, 128).T  # [128, KC]
        return np.ascontiguousarray(
            np.broadcast_to(v[:, :, None], (128, KC, BS))
        )

    fcw = np.ascontiguousarray(
        fc_w.T.reshape(KC, 128, N).transpose(1, 0, 2).astype(BF)
    )
    fcb = np.ascontiguousarray(
        np.broadcast_to(fc_b[None, :], (BS, N)).astype(np.float32)
    )

    shared = {
        "wih0": wT(W_ih0), "whh0": wT(W_hh0),
        "wih1": wT(W_ih1), "whh1": wT(W_hh1),
        "fcw": fcw,
        "bias0": bias_comb(b_ih0, b_hh0), "bias1": bias_comb(b_ih1, b_hh1),
        "bhn0": bhn(b_hh0), "bhn1": bhn(b_hh1),
        "fcb": fcb,
    }
    in_maps = []
    for core in range(NCORES):
        xb = x[core * BS : (core + 1) * BS].astype(BF)  # [BS, T, N]
        # [N, T, BS] -> [KC, 128, NBLK, TB, BS] -> [NBLK, 128, KC, TB, BS]
        xt = xb.transpose(2, 1, 0).reshape(KC, 128, T // TB, TB, BS)
        xt = np.ascontiguousarray(xt.transpose(2, 1, 0, 3, 4))[:NBLK]
        m = dict(shared)
        m["xts"] = np.ascontiguousarray(xt)
        in_maps.append(m)
    return in_maps


_NC_CACHE = {}


def _get_nc():
    if "nc" not in _NC_CACHE:
        _NC_CACHE["nc"] = _build()
    return _NC_CACHE["nc"]


def kernel(**inputs):
    nc = _get_nc()
    in_maps = _prep_inputs(**{k: np.asarray(v) for k, v in inputs.items()})
    res = run_bass_kernel_spmd(nc, in_maps, list(range(NCORES)))
    return np.concatenate([res.results[i]["y"] for i in range(NCORES)], axis=0)


if __name__ == "__main__":
    rng = np.random.default_rng(0)
    ins = {
        "x": rng.standard_normal((B, T, N), dtype=np.float32),
        "W_ih0": rng.standard_normal((3 * H, N), dtype=np.float32) * 0.04,
        "W_hh0": rng.standard_normal((3 * H, H), dtype=np.float32) * 0.04,
        "b_ih0": rng.standard_normal(3 * H).astype(np.float32) * 0.04,
        "b_hh0": rng.standard_normal(3 * H).astype(np.float32) * 0.04,
        "W_ih1": rng.standard_normal((3 * H, H), dtype=np.float32) * 0.04,
        "W_hh1": rng.standard_normal((3 * H, H), dtype=np.float32) * 0.04,
        "b_ih1": rng.standard_normal(3 * H).astype(np.float32) * 0.04,
        "b_hh1": rng.standard_normal(3 * H).astype(np.float32) * 0.04,
        "fc_w": rng.standard_normal((N, H), dtype=np.float32) * 0.04,
        "fc_b": rng.standard_normal(N).astype(np.float32) * 0.04,
    }
    out = kernel(**ins)
    print("out", out.shape, out.dtype, out.sum())
